# revision 1
# baseline (speedup 1.0000x reference)
"""Multi-head attention with bias, distributed over 8 trn2 NeuronCores.

Reference computation (per batch b):
    q = (x @ Wq.T) * depth**-0.5 ; k = y @ Wk.T ; v = y @ Wv.T     (per-head split)
    out = softmax(q @ k.T + bias) @ v @ Wo.T

Sharding: 8 cores = 4 batches x 2 query-row halves.  Core c handles batch
b = c//2 and query rows (c%2)*1024 .. +1024.  k/v projections are computed
redundantly inside each pair (25% extra flops) so there are NO collectives.

Device-side layout (everything "transposed", feature dim on partitions):
    qT/kT = W.T-projected activations [d_out, seq]; v natural [seq, d_out].
    logitsT[kk, i] = kT_h-slice.T @ qT_h-slice  (K=64 contraction)
    expw = exp(logitsT) * exp(bias).T           (exp(bias) precomputed on host)
    attnT_h(+denom row) = [v_h | ones].T @ expw (K=128, denom rides as row 64)
    normalize via batched DVE reciprocal + DMA partition-broadcast from DRAM
    outT = Wo.T-proj of normalized attnT.
Host does: transposes, bf16 casts, exp(bias), scale fold into Wq.
"""

import numpy as np
import ml_dtypes
from contextlib import ExitStack

import concourse.bass as bass
import concourse.mybir as mybir
import concourse.tile as tile
from concourse import bacc
from concourse.bass_utils import run_bass_kernel_spmd

# full-problem dims (hardcoded per spec)
B, S, D, H = 4, 2048, 1024, 16
DEPTH = D // H            # 64
P = 128
NCORES = 8

BF = mybir.dt.bfloat16
F32 = mybir.dt.float32
EXP = mybir.ActivationFunctionType.Exp

TRACE = False
last_exec_time_ns = None
last_results = None


def _chunks(total, step):
    return [(n0, min(n0 + step, total)) for n0 in range(0, total, step)]


def _attn_body(ctx, tc, io, S_, D_, H_, SL_):
    """Emit the per-core kernel.  S_: kv seq len, SL_: q rows on this core.

    Software-pipelined emission: the PE instruction stream interleaves
    v/q/k projection matmuls into the ACT-bound attention inner loop so
    the ScalarE exp stream (the bottleneck) starts early and never
    starves.  Normalization is per head-pair so only the last pair's
    reciprocal chain sits in the tail.
    """
    nc = tc.nc
    NT = D_ // P              # d tiles
    KT = S_ // P              # kk tiles
    HPT = P // DEPTH          # heads per d-tile = 2
    xT, yT, ebT, wqT, wkT, wvT, woT, outT = (
        io[k] for k in ("xT", "yT", "ebT", "wqT", "wkT", "wvT", "woT", "outT"))

    ebpool = ctx.enter_context(tc.tile_pool(name="ebpool", bufs=KT))
    qpool = ctx.enter_context(tc.tile_pool(name="qpool", bufs=2))
    kpool = ctx.enter_context(tc.tile_pool(name="kpool", bufs=2))
    vpool = ctx.enter_context(tc.tile_pool(name="vpool", bufs=KT))
    epool = ctx.enter_context(tc.tile_pool(name="epool", bufs=4))
    stpool = ctx.enter_context(tc.tile_pool(name="stpool", bufs=4))
    smpool = ctx.enter_context(tc.tile_pool(name="smpool", bufs=2))
    plp = ctx.enter_context(tc.tile_pool(name="plp", bufs=2, space="PSUM"))
    pap = ctx.enter_context(tc.tile_pool(name="pap", bufs=2, space="PSUM"))
    dpool = ctx.enter_context(tc.tile_pool(name="dpool", bufs=1, space="DRAM"))

    v_sb = [vpool.tile([P, H_, 66], BF, tag="v66", name=f"v{c}", bufs=KT)
            for c in range(KT)]
    rscr = dpool.tile([H_, SL_], BF, tag="rscr", name="rscr", bufs=1)
    audram = dpool.tile([D_, SL_], BF, tag="audram", name="audram", bufs=1)

    with tc.tile_pool(name="ypool", bufs=NT) as ypool, \
         tc.tile_pool(name="xpool", bufs=NT) as xpool, \
         tc.tile_pool(name="wqpool", bufs=NT) as wqpool, \
         tc.tile_pool(name="wvpool", bufs=NT) as wvpool, \
         tc.tile_pool(name="wkpool", bufs=NT) as wkpool:
        x_sb = [xpool.tile([P, SL_], BF, tag="xT", name=f"x{t}", bufs=NT)
                for t in range(NT)]
        for t in range(NT):
            nc.sync.dma_start(out=x_sb[t], in_=xT[t * P:(t + 1) * P, :])
        wq_sb = [wqpool.tile([P, D_], BF, tag="wq", name=f"wq{t}", bufs=NT)
                 for t in range(NT)]
        for t in range(NT):
            nc.sync.dma_start(out=wq_sb[t], in_=wqT[t * P:(t + 1) * P, :])
        y_sb = [ypool.tile([P, S_], BF, tag="yT", name=f"y{t}", bufs=NT)
                for t in range(NT)]
        for t in range(NT):
            nc.sync.dma_start(out=y_sb[t], in_=yT[t * P:(t + 1) * P, :])
        wv_sb = [wvpool.tile([P, D_], BF, tag="wv", name=f"wv{t}", bufs=NT)
                 for t in range(NT)]
        for t in range(NT):
            nc.sync.dma_start(out=wv_sb[t], in_=wvT[t * P:(t + 1) * P, :])
        wk_sb = [wkpool.tile([P, D_], BF, tag="wk", name=f"wk{t}", bufs=NT)
                 for t in range(NT)]
        for t in range(NT):
            nc.sync.dma_start(out=wk_sb[t], in_=wkT[t * P:(t + 1) * P, :])
        eb_sb = [ebpool.tile([P, SL_], BF, tag="eb", name=f"eb{c}", bufs=KT)
                 for c in range(KT)]
        for c in range(KT):
            nc.sync.dma_start(out=eb_sb[c], in_=ebT[c * P:(c + 1) * P, :])

        # warm-up heartbeats: tiny matmuls chained to arriving input DMAs
        # keep the PE HAM activity window alive through the load phase so
        # the first projections run at 2.4 GHz
        wj0 = min(512, SL_)
        jnk0 = plp.tile([1, 1024], F32, tag="pl", name="jnk0", bufs=2)
        for t in range(NT):
            nc.tensor.matmul(jnk0[0:1, 0:wj0], lhsT=x_sb[t][0:1, 0:1],
                             rhs=x_sb[t][0:1, 0:wj0], start=True, stop=True)
            nc.tensor.matmul(jnk0[0:1, 0:wj0], lhsT=y_sb[t][0:1, 0:1],
                             rhs=y_sb[t][0:1, 0:wj0], start=True, stop=True)

        # ---- emission helpers (deferred work units for pipelining) ----
        def emit_v_tile(c):
            vt = v_sb[c]
            nc.vector.memset(vt[:, :, 64:65], 1.0)
            nc.vector.memset(vt[:, :, 65:66], 0.0)
            for gi, (n0, n1) in enumerate(_chunks(D_, 512)):
                ps = plp.tile([P, 1024], F32, tag="pl", name=f"psv{c}_{gi}",
                              bufs=2)
                for u in range(NT):
                    nc.tensor.matmul(ps[:, 0:n1 - n0],
                                     lhsT=y_sb[u][:, c * P:(c + 1) * P],
                                     rhs=wv_sb[u][:, n0:n1],
                                     start=(u == 0), stop=(u == NT - 1))
                ng = (n1 - n0) // DEPTH
                src = ps[:, 0:n1 - n0].rearrange("p (g d) -> p g d", d=DEPTH)
                nc.vector.tensor_copy(vt[:, gi * ng:(gi + 1) * ng, 0:DEPTH],
                                      src)

        def emit_q_group(qt, t, n0, n1):
            ps = plp.tile([P, 1024], F32, tag="pl", name=f"psq{t}_{n0}",
                          bufs=2)
            for u in range(NT):
                nc.tensor.matmul(ps[:, 0:n1 - n0],
                                 lhsT=wq_sb[u][:, t * P:(t + 1) * P],
                                 rhs=x_sb[u][:, n0:n1],
                                 start=(u == 0), stop=(u == NT - 1))
            nc.vector.tensor_copy(qt[:, n0:n1], ps[:, 0:n1 - n0])

        def emit_k_group(kt, t, n0, n1):
            ps = plp.tile([P, 1024], F32, tag="pl", name=f"psk{t}_{n0}",
                          bufs=2)
            for u in range(NT):
                nc.tensor.matmul(ps[:, 0:n1 - n0],
                                 lhsT=wk_sb[u][:, t * P:(t + 1) * P],
                                 rhs=y_sb[u][:, n0:n1],
                                 start=(u == 0), stop=(u == NT - 1))
            nc.vector.tensor_copy(kt[:, n0:n1], ps[:, 0:n1 - n0])

        def proj_thunks(qt, kt, t):
            return ([lambda n0=n0, n1=n1: emit_q_group(qt, t, n0, n1)
                     for n0, n1 in _chunks(SL_, 512)] +
                    [lambda n0=n0, n1=n1: emit_k_group(kt, t, n0, n1)
                     for n0, n1 in _chunks(S_, 512)])

        # ---- prologue: q0/k0 projection, then first v tiles ----
        q_cur = qpool.tile([P, SL_], BF, tag="qT", name="q0", bufs=2)
        k_cur = kpool.tile([P, S_], BF, tag="kT", name="k0", bufs=2)
        for th in proj_thunks(q_cur, k_cur, 0):
            th()
        vlead = min(2, KT)
        for c in range(vlead):
            emit_v_tile(c)

        for t in range(NT):
            ha, hb = HPT * t, HPT * t + 1
            pattn = [pap.tile([65, SL_], F32, tag="pattn",
                              name=f"pa{ha + hf}", bufs=2)
                     for hf in range(HPT)]
            # deferred emissions spread across this pair's c-loop
            thunks = []
            if t == 0:
                thunks += [lambda c=c: emit_v_tile(c)
                           for c in range(vlead, KT)]
            if t + 1 < NT:
                q_nxt = qpool.tile([P, SL_], BF, tag="qT", name=f"q{t + 1}",
                                   bufs=2)
                k_nxt = kpool.tile([P, S_], BF, tag="kT", name=f"k{t + 1}",
                                   bufs=2)
                thunks += proj_thunks(q_nxt, k_nxt, t + 1)
            # schedule thunk i after c-iteration floor(i * KT / len)
            sched = {}
            for i, th in enumerate(thunks):
                sched.setdefault(i * KT // max(1, len(thunks)), []).append(th)

            for c in range(KT):
                for n0, n1 in _chunks(SL_, 512):
                    w = n1 - n0
                    plt = plp.tile([P, 1024], F32, tag="pl",
                                   name=f"pl{ha}_{c}_{n0}", bufs=2)
                    nc.tensor.matmul(plt[:, 0:w],
                                     lhsT=k_cur[0:DEPTH, c * P:(c + 1) * P],
                                     rhs=q_cur[0:DEPTH, n0:n1],
                                     start=True, stop=True)
                    nc.tensor.matmul(plt[:, w:2 * w],
                                     lhsT=k_cur[DEPTH:2 * DEPTH,
                                                c * P:(c + 1) * P],
                                     rhs=q_cur[DEPTH:2 * DEPTH, n0:n1],
                                     start=True, stop=True)
                    ew = epool.tile([P, 1024], BF, tag="ew",
                                    name=f"ew{ha}_{c}_{n0}", bufs=2)
                    nc.scalar.activation(ew[:, 0:2 * w], plt[:, 0:2 * w], EXP)
                    ew2 = epool.tile([P, 1024], BF, tag="ew2",
                                     name=f"ew2{ha}_{c}_{n0}", bufs=2)
                    nc.vector.tensor_mul(ew2[:, 0:w], ew[:, 0:w],
                                         eb_sb[c][:, n0:n1])
                    nc.vector.tensor_mul(ew2[:, w:2 * w], ew[:, w:2 * w],
                                         eb_sb[c][:, n0:n1])
                    nc.tensor.matmul(pattn[0][:, n0:n1],
                                     lhsT=v_sb[c][:, ha, 0:65],
                                     rhs=ew2[:, 0:w],
                                     start=(c == 0), stop=(c == KT - 1))
                    nc.tensor.matmul(pattn[1][:, n0:n1],
                                     lhsT=v_sb[c][:, hb, 0:65],
                                     rhs=ew2[:, w:2 * w],
                                     start=(c == 0), stop=(c == KT - 1))
                for th in sched.get(c, ()):
                    th()

            # ---- epilogue + per-pair normalization ----
            # sau rows 0-63: unnormalized attn (base partition 0); row 64:
            # denominator.  Normalize in SBUF then bounce the finished rows
            # to DRAM so no [128,SL] attn tiles stay resident.
            den_t = smpool.tile([HPT, SL_], BF, tag="dent", name=f"den{t}",
                                bufs=2)
            saus = []
            for hf in range(HPT):
                h = ha + hf
                sau = stpool.tile([65, SL_], BF, tag="sau", name=f"sa{h}",
                                  bufs=3)
                saus.append(sau)
                nc.vector.tensor_copy(sau, pattn[hf])
                nc.sync.dma_start(out=den_t[hf:hf + 1, :], in_=sau[64:65, :])
            wj = min(512, SL_)
            jnk = None
            if t == NT - 1:
                jnk = plp.tile([1, 1024], F32, tag="pl", name="jnk", bufs=2)

            def beat(ap):
                # tiny dependent matmul: keeps the PE HAM activity window
                # alive across the serial normalize tail (else the output
                # projection starts at the 1.2 GHz throttled clock)
                if jnk is not None:
                    nc.tensor.matmul(jnk[0:1, 0:wj], lhsT=ap[0:1, 0:1],
                                     rhs=ap[0:1, 0:wj], start=True, stop=True)

            denf = smpool.tile([HPT, SL_], F32, tag="denf", name=f"dnf{t}",
                               bufs=1)
            nc.vector.tensor_copy(denf, den_t)
            beat(denf)
            recipf = smpool.tile([HPT, SL_], F32, tag="recipf",
                                 name=f"rcf{t}", bufs=1)
            nc.vector.reciprocal_approx_fast(recipf, denf)
            beat(recipf)
            recipb = smpool.tile([HPT, SL_], BF, tag="recipb",
                                 name=f"rcb{t}", bufs=1)
            nc.vector.tensor_copy(recipb, recipf)
            nc.sync.dma_start(out=rscr[HPT * t:HPT * (t + 1), :], in_=recipb)
            for hf in range(HPT):
                h = ha + hf
                bc = smpool.tile([DEPTH, SL_], BF, tag="bc", name=f"bc{h}",
                                 bufs=1)
                nc.sync.dma_start(
                    out=bc, in_=rscr[h:h + 1, :].partition_broadcast(DEPTH))
                beat(bc)
                anh = smpool.tile([DEPTH, SL_], BF, tag="anh", name=f"an{h}",
                                  bufs=2)
                nc.vector.tensor_mul(anh, saus[hf][0:64, :], bc)
                beat(anh)
                nc.sync.dma_start(
                    out=audram[t * P + hf * DEPTH:t * P + (hf + 1) * DEPTH, :],
                    in_=anh)
            if t + 1 < NT:
                q_cur, k_cur = q_nxt, k_nxt

    # ---------------- output projection ----------------
    opool = ctx.enter_context(tc.tile_pool(name="opool", bufs=2))
    wopool = ctx.enter_context(tc.tile_pool(name="wopool", bufs=NT))
    ropool = ctx.enter_context(tc.tile_pool(name="ropool", bufs=NT))
    wo_sb = [wopool.tile([P, D_], BF, tag="wo", name=f"wo{t}", bufs=NT)
             for t in range(NT)]
    for t in range(NT):
        nc.gpsimd.dma_start(out=wo_sb[t], in_=woT[t * P:(t + 1) * P, :])
    an_sb = [ropool.tile([P, SL_], BF, tag="an", name=f"ran{t}", bufs=NT)
             for t in range(NT)]
    for t in range(NT):
        nc.gpsimd.dma_start(out=an_sb[t], in_=audram[t * P:(t + 1) * P, :])
    jnk2 = plp.tile([1, 1024], F32, tag="pl", name="jnk2", bufs=2)
    wj = min(512, SL_)
    for t in (0, NT - 1):
        nc.tensor.matmul(jnk2[0:1, 0:wj], lhsT=an_sb[t][0:1, 0:1],
                         rhs=an_sb[t][0:1, 0:wj], start=True, stop=True)
    for m in range(NT):
        osb = opool.tile([P, SL_], F32, tag="osb", name=f"o{m}", bufs=2)
        for n0, n1 in _chunks(SL_, 512):
            ps = plp.tile([P, 1024], F32, tag="pl", name=f"pso{m}_{n0}", bufs=2)
            for t in range(NT):
                nc.tensor.matmul(ps[:, 0:n1 - n0],
                                 lhsT=wo_sb[t][:, m * P:(m + 1) * P],
                                 rhs=an_sb[t][:, n0:n1],
                                 start=(t == 0), stop=(t == NT - 1))
            nc.vector.tensor_copy(osb[:, n0:n1], ps[:, 0:n1 - n0])
        nc.sync.dma_start(out=outT[m * P:(m + 1) * P, :], in_=osb)


def build_nc(S_=S, D_=D, H_=H, SL_=None):
    if SL_ is None:
        SL_ = S_ // 2
    nc = bacc.Bacc("TRN2", target_bir_lowering=False, debug=False)
    io = {
        "xT": nc.dram_tensor("xT", [D_, SL_], BF, kind="ExternalInput").ap(),
        "yT": nc.dram_tensor("yT", [D_, S_], BF, kind="ExternalInput").ap(),
        "ebT": nc.dram_tensor("ebT", [S_, SL_], BF, kind="ExternalInput").ap(),
        "wqT": nc.dram_tensor("wqT", [D_, D_], BF, kind="ExternalInput").ap(),
        "wkT": nc.dram_tensor("wkT", [D_, D_], BF, kind="ExternalInput").ap(),
        "wvT": nc.dram_tensor("wvT", [D_, D_], BF, kind="ExternalInput").ap(),
        "woT": nc.dram_tensor("woT", [D_, D_], BF, kind="ExternalInput").ap(),
        "outT": nc.dram_tensor("outT", [D_, SL_], F32,
                               kind="ExternalOutput").ap(),
    }
    with tile.TileContext(nc) as tc:
        with ExitStack() as ctx:
            _attn_body(ctx, tc, io, S_, D_, H_, SL_)
    nc.compile()
    return nc


_NC_CACHE = None


def kernel(x, y, bias, Wq, Wk, Wv, Wo):
    global _NC_CACHE, last_exec_time_ns, last_results
    x = np.asarray(x, np.float32)
    y = np.asarray(y, np.float32)
    bias = np.asarray(bias, np.float32)
    Wq, Wk, Wv, Wo = (np.asarray(w, np.float32) for w in (Wq, Wk, Wv, Wo))
    SL_ = S // 2
    if _NC_CACHE is None:
        _NC_CACHE = build_nc()
    nc = _NC_CACHE

    bf = ml_dtypes.bfloat16
    scale = DEPTH ** -0.5
    wqT = np.ascontiguousarray(Wq.T * scale).astype(bf)
    wkT = np.ascontiguousarray(Wk.T).astype(bf)
    wvT = np.ascontiguousarray(Wv.T).astype(bf)
    woT = np.ascontiguousarray(Wo.T).astype(bf)
    eb = np.exp(bias[0, 0].astype(np.float32))
    ebT_half = [np.ascontiguousarray(eb[q0:q0 + SL_, :].T).astype(bf)
                for q0 in (0, SL_)]
    yT_all = [np.ascontiguousarray(y[b].T).astype(bf) for b in range(B)]

    in_maps = []
    for core in range(NCORES):
        b, half = divmod(core, 2)
        qs = half * SL_
        in_maps.append({
            "xT": np.ascontiguousarray(x[b, qs:qs + SL_, :].T).astype(bf),
            "yT": yT_all[b],
            "ebT": ebT_half[half],
            "wqT": wqT, "wkT": wkT, "wvT": wvT, "woT": woT,
        })

    res = run_bass_kernel_spmd(nc, in_maps, core_ids=list(range(NCORES)),
                               trace=TRACE)
    last_exec_time_ns = res.exec_time_ns
    last_results = res
    out = np.empty((B, S, D), np.float32)
    for core in range(NCORES):
        b, half = divmod(core, 2)
        qs = half * SL_
        out[b, qs:qs + SL_, :] = res.results[core]["outT"].T
    return out



# revision 4
# speedup vs baseline: 1.1585x; 1.1585x over previous
"""Multi-head attention with bias, distributed over 8 trn2 NeuronCores.

Reference computation (per batch b):
    q = (x @ Wq.T) * depth**-0.5 ; k = y @ Wk.T ; v = y @ Wv.T     (per-head split)
    out = softmax(q @ k.T + bias) @ v @ Wo.T

Sharding: 8 cores = 4 batches x 2 head-halves (tensor parallel over heads).
Core c handles batch b = c//2 and heads (c%2)*8 .. +8.  Wq/Wk/Wv are
column-split, Wo row-split; the two partial outputs per batch are summed on
the host (no device collective).

Device-side layout (feature dim on partitions):
    qT/kT = W.T-projected activations [d_out=512, 2048]; v natural [kk, h, d].
    logitsT[kk, q] per head via row-tiled K=64 matmul pairs (2 heads share
    the 128-partition d-tile; tile_position rows 0-63 / 64-127 concurrent)
    expw = exp(logitsT) * exp(bias).T      (exp(bias) precomputed on host,
                                            streamed per (pair, q-chunk))
    attnT_h(+denom row) = [v_h | ones].T @ expw  (K=128, denom rides row 64)
    normalize via DVE reciprocal + DMA partition-broadcast from DRAM
    outT_partial = Wo_half.T-proj of normalized attnT (summed on host).
Host does: transposes, bf16 casts, exp(bias) pre-tiling, scale fold into Wq.
"""

import numpy as np
import ml_dtypes
from contextlib import ExitStack

import concourse.bass as bass
import concourse.mybir as mybir
import concourse.tile as tile
from concourse import bacc
from concourse.bass_utils import run_bass_kernel_spmd

# full-problem dims (hardcoded per spec)
B, S, D, H = 4, 2048, 1024, 16
DEPTH = D // H            # 64
P = 128
NCORES = 8

DH = D // 2               # 512 head dims per core (8 heads)
NPAIR = 4                 # head pairs per core
NCH = 4                   # q chunks of 512
KT = S // P               # 16 kk tiles
NU = D // P               # 8 d_in tiles
CW = 512                  # q chunk width

BF = mybir.dt.bfloat16
F32 = mybir.dt.float32
EXP = mybir.ActivationFunctionType.Exp

TRACE = False
last_exec_time_ns = None
last_results = None


def _attn_body(ctx, tc, io):
    nc = tc.nc
    xT, yT, wqT, wkT, wvT, woT, ebt, outT = (
        io[k] for k in ("xT", "yT", "wqT", "wkT", "wvT", "woT", "ebt", "outT"))

    # ---------------- persistent pools ----------------
    qpool = ctx.enter_context(tc.tile_pool(name="qpool", bufs=NPAIR))
    kpool = ctx.enter_context(tc.tile_pool(name="kpool", bufs=NPAIR))
    vpool = ctx.enter_context(tc.tile_pool(name="vpool", bufs=KT))
    anpool = ctx.enter_context(tc.tile_pool(name="anpool", bufs=NPAIR))
    ebpool = ctx.enter_context(tc.tile_pool(name="ebpool", bufs=24))
    epool = ctx.enter_context(tc.tile_pool(name="epool", bufs=4))
    smpool = ctx.enter_context(tc.tile_pool(name="smpool", bufs=4))
    plp = ctx.enter_context(tc.tile_pool(name="plp", bufs=2, space="PSUM"))
    pap = ctx.enter_context(tc.tile_pool(name="pap", bufs=2, space="PSUM"))
    pop = ctx.enter_context(tc.tile_pool(name="pop", bufs=2, space="PSUM"))
    dpool = ctx.enter_context(tc.tile_pool(name="dpool", bufs=2, space="DRAM"))

    qT_sb = [qpool.tile([P, S], BF, tag="qT", name=f"qT{p}", bufs=NPAIR)
             for p in range(NPAIR)]
    kT_sb = [kpool.tile([P, S], BF, tag="kT", name=f"kT{p}", bufs=NPAIR)
             for p in range(NPAIR)]
    v_sb = [vpool.tile([P, 2 * NPAIR, 66], BF, tag="v66", name=f"v{c}",
                       bufs=KT) for c in range(KT)]
    an_sb = [anpool.tile([P, S], BF, tag="an", name=f"an{p}", bufs=NPAIR)
             for p in range(NPAIR)]

    def eb_tile(p, ch, c):
        return ebpool.tile([P, CW], BF, tag="eb", name=f"eb{p}_{ch}_{c}",
                           bufs=24)

    def dma_eb_slab(ch, tiles):
        base = ch * S
        for c in range(KT):
            nc.sync.dma_start(out=tiles[c],
                              in_=ebt[base + c * P:base + (c + 1) * P, :])

    state = {"eb_cur": None}

    def run_pair(p, sched):
        """Emit one head-pair's attention (all 4 q-chunks, 16 kk tiles)."""
        ha, hb = 2 * p, 2 * p + 1
        for ch in range(NCH):
            # prefetch next eb slab
            np_, nch = (p, ch + 1) if ch + 1 < NCH else (p + 1, 0)
            eb_nxt = None
            if np_ < NPAIR:
                eb_nxt = [eb_tile(np_, nch, c) for c in range(KT)]
                dma_eb_slab(nch, eb_nxt)
            eb_cur = state["eb_cur"]
            pattn = [pap.tile([65, CW], F32, tag="pattn",
                              name=f"pa{p}_{ch}_{hf}", bufs=2)
                     for hf in range(2)]
            for c in range(KT):
                plt = plp.tile([P, 2 * CW], F32, tag="pl",
                               name=f"pl{p}_{ch}_{c}", bufs=2)
                nc.tensor.matmul(plt[:, 0:CW],
                                 lhsT=kT_sb[p][0:DEPTH, c * P:(c + 1) * P],
                                 rhs=qT_sb[p][0:DEPTH, ch * CW:(ch + 1) * CW],
                                 start=True, stop=True)
                nc.tensor.matmul(plt[:, CW:2 * CW],
                                 lhsT=kT_sb[p][DEPTH:2 * DEPTH,
                                               c * P:(c + 1) * P],
                                 rhs=qT_sb[p][DEPTH:2 * DEPTH,
                                              ch * CW:(ch + 1) * CW],
                                 start=True, stop=True)
                ew = epool.tile([P, 2 * CW], BF, tag="ew",
                                name=f"ew{p}_{ch}_{c}", bufs=2)
                nc.scalar.activation(ew, plt, EXP)
                ew2 = epool.tile([P, 2 * CW], BF, tag="ew2",
                                 name=f"ew2{p}_{ch}_{c}", bufs=2)
                nc.vector.tensor_mul(ew2[:, 0:CW], ew[:, 0:CW], eb_cur[c])
                nc.vector.tensor_mul(ew2[:, CW:2 * CW], ew[:, CW:2 * CW],
                                     eb_cur[c])
                nc.tensor.matmul(pattn[0], lhsT=v_sb[c][:, ha, 0:65],
                                 rhs=ew2[:, 0:CW],
                                 start=(c == 0), stop=(c == KT - 1))
                nc.tensor.matmul(pattn[1], lhsT=v_sb[c][:, hb, 0:65],
                                 rhs=ew2[:, CW:2 * CW],
                                 start=(c == 0), stop=(c == KT - 1))
                for th in sched.get(ch * KT + c, ()):
                    th()

            # ---- normalization epilogue for (p, ch) ----
            saus = []
            for hf in range(2):
                sau = smpool.tile([65, CW], BF, tag="sau",
                                  name=f"sa{p}_{ch}_{hf}", bufs=3)
                saus.append(sau)
                nc.vector.tensor_copy(sau, pattn[hf])
            den_t = smpool.tile([2, CW], BF, tag="dent",
                                name=f"den{p}_{ch}", bufs=2)
            for hf in range(2):
                nc.gpsimd.dma_start(out=den_t[hf:hf + 1, :],
                                    in_=saus[hf][64:65, :])
            denf = smpool.tile([2, CW], F32, tag="denf",
                               name=f"dnf{p}_{ch}", bufs=2)
            nc.vector.tensor_copy(denf, den_t)
            recipf = smpool.tile([2, CW], F32, tag="recipf",
                                 name=f"rcf{p}_{ch}", bufs=2)
            nc.vector.reciprocal_approx_fast(recipf, denf)
            recipb = smpool.tile([2, CW], BF, tag="recipb",
                                 name=f"rcb{p}_{ch}", bufs=2)
            nc.vector.tensor_copy(recipb, recipf)
            rscr = dpool.tile([2, CW], BF, tag="rscr",
                              name=f"rs{p}_{ch}", bufs=2)
            nc.gpsimd.dma_start(out=rscr, in_=recipb)
            for hf in range(2):
                bc = smpool.tile([DEPTH, CW], BF, tag="bc",
                                 name=f"bc{p}_{ch}_{hf}", bufs=2)
                nc.gpsimd.dma_start(
                    out=bc, in_=rscr[hf:hf + 1, :].partition_broadcast(DEPTH))
                nc.vector.tensor_mul(
                    an_sb[p][hf * DEPTH:(hf + 1) * DEPTH,
                             ch * CW:(ch + 1) * CW],
                    saus[hf][0:DEPTH, :], bc)
            state["eb_cur"] = eb_nxt

    def make_sched(thunks):
        sched = {}
        for i, th in enumerate(thunks):
            it = i * (NCH * KT) // max(1, len(thunks))
            sched.setdefault(it, []).append(th)
        return sched

    # ---------------- load + projection phase (pairs 0-1) ----------------
    with tc.tile_pool(name="ypool", bufs=NU) as ypool, \
         tc.tile_pool(name="xpool", bufs=NU) as xpool, \
         tc.tile_pool(name="wkpool", bufs=NU) as wkpool, \
         tc.tile_pool(name="wqpool", bufs=NU) as wqpool, \
         tc.tile_pool(name="wvpool", bufs=NU) as wvpool:
        wk_sb = [wkpool.tile([P, DH], BF, tag="wk", name=f"wk{u}", bufs=NU)
                 for u in range(NU)]
        y_sb = [ypool.tile([P, S], BF, tag="yT", name=f"y{u}", bufs=NU)
                for u in range(NU)]
        x_sb = [xpool.tile([P, S], BF, tag="xT", name=f"x{u}", bufs=NU)
                for u in range(NU)]
        wq_sb = [wqpool.tile([P, DH], BF, tag="wq", name=f"wq{u}", bufs=NU)
                 for u in range(NU)]
        wv_sb = [wvpool.tile([P, DH], BF, tag="wv", name=f"wv{u}", bufs=NU)
                 for u in range(NU)]
        for u in range(NU):
            nc.sync.dma_start(out=wk_sb[u], in_=wkT[u * P:(u + 1) * P, :])
            nc.sync.dma_start(out=y_sb[u], in_=yT[u * P:(u + 1) * P, :])
        for u in range(NU):
            nc.sync.dma_start(out=wv_sb[u], in_=wvT[u * P:(u + 1) * P, :])
            nc.sync.dma_start(out=x_sb[u], in_=xT[u * P:(u + 1) * P, :])
            nc.sync.dma_start(out=wq_sb[u], in_=wqT[u * P:(u + 1) * P, :])
        eb0 = [eb_tile(0, 0, c) for c in range(KT)]
        dma_eb_slab(0, eb0)
        state["eb_cur"] = eb0

        # warm-up heartbeats: tiny matmuls chained to arriving input DMAs
        # keep the PE HAM activity window alive through the load phase
        jnk0 = plp.tile([1, 1024], F32, tag="pl", name="jnk0", bufs=2)
        for u in range(NU):
            nc.tensor.matmul(jnk0[0:1, 0:512], lhsT=y_sb[u][0:1, 0:1],
                             rhs=y_sb[u][0:1, 0:512], start=True, stop=True)
            nc.tensor.matmul(jnk0[0:1, 0:512], lhsT=x_sb[u][0:1, 0:1],
                             rhs=x_sb[u][0:1, 0:512], start=True, stop=True)

        # ---- emission helpers (deferred work units for pipelining) ----
        def emit_k_group(p, kkc):
            ps = pop.tile([P, CW], F32, tag="po", name=f"psk{p}_{kkc}", bufs=2)
            for u in range(NU):
                nc.tensor.matmul(ps, lhsT=wk_sb[u][:, p * P:(p + 1) * P],
                                 rhs=y_sb[u][:, kkc * CW:(kkc + 1) * CW],
                                 start=(u == 0), stop=(u == NU - 1))
            nc.vector.tensor_copy(kT_sb[p][:, kkc * CW:(kkc + 1) * CW], ps)

        def emit_q_group(p, ch):
            ps = pop.tile([P, CW], F32, tag="po", name=f"psq{p}_{ch}", bufs=2)
            for u in range(NU):
                nc.tensor.matmul(ps, lhsT=wq_sb[u][:, p * P:(p + 1) * P],
                                 rhs=x_sb[u][:, ch * CW:(ch + 1) * CW],
                                 start=(u == 0), stop=(u == NU - 1))
            nc.vector.tensor_copy(qT_sb[p][:, ch * CW:(ch + 1) * CW], ps)

        def emit_v_tile(c):
            vt = v_sb[c]
            nc.vector.memset(vt[:, :, 64:65], 1.0)
            nc.vector.memset(vt[:, :, 65:66], 0.0)
            ps = pop.tile([P, CW], F32, tag="po", name=f"psv{c}", bufs=2)
            for u in range(NU):
                nc.tensor.matmul(ps, lhsT=y_sb[u][:, c * P:(c + 1) * P],
                                 rhs=wv_sb[u][:, 0:DH],
                                 start=(u == 0), stop=(u == NU - 1))
            src = ps.rearrange("p (g d) -> p g d", d=DEPTH)
            nc.vector.tensor_copy(vt[:, :, 0:DEPTH], src)

        # ---- prologue: pair-0 k, q chunk 0, first v tiles ----
        for kkc in range(NCH):
            emit_k_group(0, kkc)
        emit_q_group(0, 0)
        for c in range(2):
            emit_v_tile(c)

        th0 = ([lambda c=c: emit_v_tile(c) for c in range(2, KT)] +
               [lambda ch=ch: emit_q_group(0, ch) for ch in range(1, NCH)] +
               [lambda kkc=kkc: emit_k_group(1, kkc) for kkc in range(NCH)] +
               [lambda ch=ch: emit_q_group(1, ch) for ch in range(NCH)])
        th1 = ([lambda kkc=kkc: emit_k_group(2, kkc) for kkc in range(NCH)] +
               [lambda ch=ch: emit_q_group(2, ch) for ch in range(NCH)] +
               [lambda kkc=kkc: emit_k_group(3, kkc) for kkc in range(NCH)] +
               [lambda ch=ch: emit_q_group(3, ch) for ch in range(NCH)])

        run_pair(0, make_sched(th0))
        run_pair(1, make_sched(th1))

    # ------------- pairs 2-3 (x/y/w pools freed), out-projection -------------
    wopool = ctx.enter_context(tc.tile_pool(name="wopool", bufs=NPAIR))
    opool = ctx.enter_context(tc.tile_pool(name="opool", bufs=2))
    wo_sb = [wopool.tile([P, D], BF, tag="wo", name=f"wo{p}", bufs=NPAIR)
             for p in range(NPAIR)]

    def load_wo(p):
        nc.gpsimd.dma_start(out=wo_sb[p], in_=woT[p * P:(p + 1) * P, :])

    def emit_out_group(m, ch):
        ps = pop.tile([P, CW], F32, tag="po", name=f"pso{m}_{ch}", bufs=2)
        for p4 in range(NPAIR):
            nc.tensor.matmul(ps, lhsT=wo_sb[p4][:, m * P:(m + 1) * P],
                             rhs=an_sb[p4][:, ch * CW:(ch + 1) * CW],
                             start=(p4 == 0), stop=(p4 == NPAIR - 1))
        osb = opool.tile([P, CW], F32, tag="osb", name=f"o{m}_{ch}", bufs=2)
        nc.vector.tensor_copy(osb, ps)
        nc.gpsimd.dma_start(
            out=outT[m * P:(m + 1) * P, ch * CW:(ch + 1) * CW], in_=osb)

    run_pair(2, make_sched([lambda p=p: load_wo(p) for p in range(NPAIR)]))

    # pair 3: out-proj for chunk ch interleaved into chunk ch+1's c-loop
    sched3 = {}
    for ch in range(NCH - 1):
        for m in range(NU):
            it = (ch + 1) * KT + 2 * m
            sched3.setdefault(it, []).append(
                lambda m=m, ch=ch: emit_out_group(m, ch))
    run_pair(3, sched3)

    # tail: last chunk's out-projection
    for m in range(NU):
        emit_out_group(m, NCH - 1)


def build_nc():
    nc = bacc.Bacc("TRN2", target_bir_lowering=False, debug=False)
    io = {
        "xT": nc.dram_tensor("xT", [D, S], BF, kind="ExternalInput").ap(),
        "yT": nc.dram_tensor("yT", [D, S], BF, kind="ExternalInput").ap(),
        "wqT": nc.dram_tensor("wqT", [D, DH], BF, kind="ExternalInput").ap(),
        "wkT": nc.dram_tensor("wkT", [D, DH], BF, kind="ExternalInput").ap(),
        "wvT": nc.dram_tensor("wvT", [D, DH], BF, kind="ExternalInput").ap(),
        "woT": nc.dram_tensor("woT", [DH, D], BF, kind="ExternalInput").ap(),
        "ebt": nc.dram_tensor("ebt", [NCH * S, CW], BF,
                              kind="ExternalInput").ap(),
        "outT": nc.dram_tensor("outT", [D, S], F32,
                               kind="ExternalOutput").ap(),
    }
    with tile.TileContext(nc) as tc:
        with ExitStack() as ctx:
            _attn_body(ctx, tc, io)
    nc.compile()
    return nc


_NC_CACHE = None


def kernel(x, y, bias, Wq, Wk, Wv, Wo):
    global _NC_CACHE, last_exec_time_ns, last_results
    x = np.asarray(x, np.float32)
    y = np.asarray(y, np.float32)
    bias = np.asarray(bias, np.float32)
    Wq, Wk, Wv, Wo = (np.asarray(w, np.float32) for w in (Wq, Wk, Wv, Wo))
    if _NC_CACHE is None:
        _NC_CACHE = build_nc()
    nc = _NC_CACHE

    bf = ml_dtypes.bfloat16
    scale = DEPTH ** -0.5
    wqT = np.ascontiguousarray(Wq.T * scale).astype(bf)
    wkT = np.ascontiguousarray(Wk.T).astype(bf)
    wvT = np.ascontiguousarray(Wv.T).astype(bf)
    woT = np.ascontiguousarray(Wo.T).astype(bf)

    # exp(bias).T pre-tiled: row ch*S + c*128 + p  <-  ebT[c*128+p, ch*512:+512]
    ebT = np.exp(bias[0, 0].astype(np.float32)).T
    ebt = np.ascontiguousarray(
        ebT.reshape(S, NCH, CW).transpose(1, 0, 2).reshape(NCH * S, CW)
    ).astype(bf)

    yT_all = [np.ascontiguousarray(y[b].T).astype(bf) for b in range(B)]
    xT_all = [np.ascontiguousarray(x[b].T).astype(bf) for b in range(B)]
    whalf = []
    for h in range(2):
        sl = slice(h * DH, (h + 1) * DH)
        whalf.append({
            "wqT": np.ascontiguousarray(wqT[:, sl]),
            "wkT": np.ascontiguousarray(wkT[:, sl]),
            "wvT": np.ascontiguousarray(wvT[:, sl]),
            "woT": np.ascontiguousarray(woT[sl, :]),
        })

    in_maps = []
    for core in range(NCORES):
        b, half = divmod(core, 2)
        m = {"xT": xT_all[b], "yT": yT_all[b], "ebt": ebt}
        m.update(whalf[half])
        in_maps.append(m)

    res = run_bass_kernel_spmd(nc, in_maps, core_ids=list(range(NCORES)),
                               trace=TRACE)
    last_exec_time_ns = res.exec_time_ns
    last_results = res
    out = np.empty((B, S, D), np.float32)
    for b in range(B):
        acc = res.results[2 * b]["outT"] + res.results[2 * b + 1]["outT"]
        out[b] = acc.T
    return out


# revision 8
# speedup vs baseline: 1.1982x; 1.0343x over previous
"""Multi-head attention with bias, distributed over 8 trn2 NeuronCores.

Reference computation (per batch b):
    q = (x @ Wq.T) * depth**-0.5 ; k = y @ Wk.T ; v = y @ Wv.T     (per-head split)
    out = softmax(q @ k.T + bias) @ v @ Wo.T

Sharding: 8 cores = 4 batches x 2 head-halves (tensor parallel over heads).
Core c handles batch b = c//2 and heads (c%2)*8 .. +8.  Wq/Wk/Wv are
column-split, Wo row-split; the two partial outputs per batch are summed on
the host (no device collective).

Device-side layout (feature dim on partitions):
    qT/kT = W.T-projected activations [d_out=512, 2048]; v natural [kk, h, d].
    logitsT[kk, q] per head via row-tiled K=64 matmul pairs (2 heads share
    the 128-partition d-tile; tile_position rows 0-63 / 64-127 concurrent)
    expw = exp(logitsT) * exp(bias).T      (exp(bias) precomputed on host,
                                            streamed per (pair, q-chunk))
    attnT_h(+denom row) = [v_h | ones].T @ expw  (K=128, denom rides row 64)
    normalize via DVE reciprocal + DMA partition-broadcast from DRAM
    outT_partial = Wo_half.T-proj of normalized attnT (summed on host).

Scheduling: the attention inner loop is emitted with the attn matmuls LAGGED
two iterations behind their logits pair so the in-order PE queue never
blocks on the ACT->DVE chain; projection matmuls are chopped into per-MM
filler ops drained ~2 per iteration into the PE slack.
Host does: transposes, bf16 casts, exp(bias) pre-tiling, scale fold into Wq.
"""

import numpy as np
import ml_dtypes
from collections import deque
from contextlib import ExitStack

import concourse.bass as bass
import concourse.mybir as mybir
import concourse.tile as tile
from concourse import bacc
from concourse.bass_utils import run_bass_kernel_spmd

# full-problem dims (hardcoded per spec)
B, S, D, H = 4, 2048, 1024, 16
DEPTH = D // H            # 64
P = 128
NCORES = 8

DH = D // 2               # 512 head dims per core (8 heads)
NPAIR = 4                 # head pairs per core
NCH = 4                   # q chunks of 512
KT = S // P               # 16 kk tiles
NU = D // P               # 8 d_in tiles
CW = 512                  # q chunk width
LAG = 2                   # attn matmul lag (iterations) behind logits

BF = mybir.dt.bfloat16
F32 = mybir.dt.float32
EXP = mybir.ActivationFunctionType.Exp

TRACE = False
last_exec_time_ns = None
last_results = None


def _attn_body(ctx, tc, io):
    nc = tc.nc
    xT, yT, wqT, wkT, wvT, woT, ebt, outT = (
        io[k] for k in ("xT", "yT", "wqT", "wkT", "wvT", "woT", "ebt", "outT"))

    # ---------------- persistent pools ----------------
    qpool = ctx.enter_context(tc.tile_pool(name="qpool", bufs=NPAIR))
    kpool = ctx.enter_context(tc.tile_pool(name="kpool", bufs=NPAIR))
    vpool = ctx.enter_context(tc.tile_pool(name="vpool", bufs=KT))
    anpool = ctx.enter_context(tc.tile_pool(name="anpool", bufs=NPAIR))
    ebpool = ctx.enter_context(tc.tile_pool(name="ebpool", bufs=20))
    epool = ctx.enter_context(tc.tile_pool(name="epool", bufs=6))
    smpool = ctx.enter_context(tc.tile_pool(name="smpool", bufs=4))
    plp = ctx.enter_context(tc.tile_pool(name="plp", bufs=2, space="PSUM"))
    pap = ctx.enter_context(tc.tile_pool(name="pap", bufs=2, space="PSUM"))
    pop = ctx.enter_context(tc.tile_pool(name="pop", bufs=2, space="PSUM"))
    dpool = ctx.enter_context(tc.tile_pool(name="dpool", bufs=2, space="DRAM"))
    wopool = ctx.enter_context(tc.tile_pool(name="wopool", bufs=NPAIR))
    opool = ctx.enter_context(tc.tile_pool(name="opool", bufs=2))

    qT_sb = [qpool.tile([P, S], BF, tag="qT", name=f"qT{p}", bufs=NPAIR)
             for p in range(NPAIR)]
    kT_sb = [kpool.tile([P, S], BF, tag="kT", name=f"kT{p}", bufs=NPAIR)
             for p in range(NPAIR)]
    v_sb = [vpool.tile([P, 2 * NPAIR, 66], BF, tag="v66", name=f"v{c}",
                       bufs=KT) for c in range(KT)]
    an_sb = [anpool.tile([P, S], BF, tag="an", name=f"an{p}", bufs=NPAIR)
             for p in range(NPAIR)]

    def eb_tile(p, ch, c):
        return ebpool.tile([P, CW], BF, tag="eb", name=f"eb{p}_{ch}_{c}",
                           bufs=20)

    def dma_eb_slab(ch, tiles):
        base = ch * S
        for c in range(KT):
            nc.sync.dma_start(out=tiles[c],
                              in_=ebt[base + c * P:base + (c + 1) * P, :])

    # ---------------- input loads + projection helpers ----------------
    with tc.tile_pool(name="ypool", bufs=NU) as ypool, \
         tc.tile_pool(name="xpool", bufs=NU) as xpool, \
         tc.tile_pool(name="wkpool", bufs=NU) as wkpool, \
         tc.tile_pool(name="wqpool", bufs=NU) as wqpool, \
         tc.tile_pool(name="wvpool", bufs=NU) as wvpool:
        wk_sb = [wkpool.tile([P, DH], BF, tag="wk", name=f"wk{u}", bufs=NU)
                 for u in range(NU)]
        y_sb = [ypool.tile([P, S], BF, tag="yT", name=f"y{u}", bufs=NU)
                for u in range(NU)]
        x_sb = [xpool.tile([P, S], BF, tag="xT", name=f"x{u}", bufs=NU)
                for u in range(NU)]
        wq_sb = [wqpool.tile([P, DH], BF, tag="wq", name=f"wq{u}", bufs=NU)
                 for u in range(NU)]
        wv_sb = [wvpool.tile([P, DH], BF, tag="wv", name=f"wv{u}", bufs=NU)
                 for u in range(NU)]
        # sync queue: wk, y, wv (k/v projection inputs)
        for u in range(NU):
            nc.sync.dma_start(out=wk_sb[u], in_=wkT[u * P:(u + 1) * P, :])
        for u in range(NU):
            nc.sync.dma_start(out=y_sb[u], in_=yT[u * P:(u + 1) * P, :])
            nc.sync.dma_start(out=wv_sb[u], in_=wvT[u * P:(u + 1) * P, :])
        # gpsimd queue (parallel channel): wq, x, first eb slab
        for u in range(NU):
            nc.gpsimd.dma_start(out=wq_sb[u], in_=wqT[u * P:(u + 1) * P, :])
        for u in range(NU):
            nc.gpsimd.dma_start(out=x_sb[u], in_=xT[u * P:(u + 1) * P, :])
        eb0 = [eb_tile(0, 0, c) for c in range(KT)]
        for c in range(KT):
            nc.gpsimd.dma_start(out=eb0[c], in_=ebt[c * P:(c + 1) * P, :])

        # warm-up heartbeats: tiny matmuls chained to arriving input DMAs
        # keep the PE HAM activity window alive through the load phase
        jnk0 = plp.tile([1, 1024], F32, tag="pl", name="jnk0", bufs=2)
        for u in range(NU):
            nc.tensor.matmul(jnk0[0:1, 0:512], lhsT=y_sb[u][0:1, 0:1],
                             rhs=y_sb[u][0:1, 0:512], start=True, stop=True)
            nc.tensor.matmul(jnk0[0:1, 0:512], lhsT=x_sb[u][0:1, 0:1],
                             rhs=x_sb[u][0:1, 0:512], start=True, stop=True)

        # ---- per-MM filler ops (consumed ~2 per attention iteration) ----
        def k_group_ops(p, kkc):
            box = {}
            def mm(u, box=box):
                if u == 0:
                    box["ps"] = pop.tile([P, CW], F32, tag="po",
                                         name=f"psk{p}_{kkc}", bufs=2)
                nc.tensor.matmul(box["ps"],
                                 lhsT=wk_sb[u][:, p * P:(p + 1) * P],
                                 rhs=y_sb[u][:, kkc * CW:(kkc + 1) * CW],
                                 start=(u == 0), stop=(u == NU - 1))
            def fin(box=box):
                nc.vector.tensor_copy(
                    kT_sb[p][:, kkc * CW:(kkc + 1) * CW], box["ps"])
            return [lambda u=u, mm=mm: mm(u) for u in range(NU)] + [fin]

        def q_group_ops(p, ch):
            box = {}
            def mm(u, box=box):
                if u == 0:
                    box["ps"] = pop.tile([P, CW], F32, tag="po",
                                         name=f"psq{p}_{ch}", bufs=2)
                nc.tensor.matmul(box["ps"],
                                 lhsT=wq_sb[u][:, p * P:(p + 1) * P],
                                 rhs=x_sb[u][:, ch * CW:(ch + 1) * CW],
                                 start=(u == 0), stop=(u == NU - 1))
            def fin(box=box):
                nc.vector.tensor_copy(
                    qT_sb[p][:, ch * CW:(ch + 1) * CW], box["ps"])
            return [lambda u=u, mm=mm: mm(u) for u in range(NU)] + [fin]

        def v_group_ops(c, h0, nh):
            """Project v for heads [h0, h0+nh) of kk-tile c (N = nh*64)."""
            box = {}
            w = nh * DEPTH
            def mm(u, box=box):
                if u == 0:
                    box["ps"] = pop.tile([P, CW], F32, tag="po",
                                         name=f"psv{c}_{h0}", bufs=2)
                nc.tensor.matmul(box["ps"][:, 0:w],
                                 lhsT=y_sb[u][:, c * P:(c + 1) * P],
                                 rhs=wv_sb[u][:, h0 * DEPTH:h0 * DEPTH + w],
                                 start=(u == 0), stop=(u == NU - 1))
            def fin(box=box):
                vt = v_sb[c]
                nc.vector.memset(vt[:, h0:h0 + nh, 64:65], 1.0)
                nc.vector.memset(vt[:, h0:h0 + nh, 65:66], 0.0)
                src = box["ps"][:, 0:w].rearrange("p (g d) -> p g d", d=DEPTH)
                nc.vector.tensor_copy(vt[:, h0:h0 + nh, 0:DEPTH], src)
            return [lambda u=u, mm=mm: mm(u) for u in range(NU)] + [fin]

        # ---------------- out-projection ----------------
        wo_sb = [wopool.tile([P, D], BF, tag="wo", name=f"wo{p}",
                             bufs=NPAIR) for p in range(NPAIR)]

        def load_wo(p):
            nc.gpsimd.dma_start(out=wo_sb[p], in_=woT[p * P:(p + 1) * P, :])

        def out_group_ops(m, ch):
            box = {}
            def mm(p4, box=box):
                if p4 == 0:
                    box["ps"] = pop.tile([P, CW], F32, tag="po",
                                         name=f"pso{m}_{ch}", bufs=2)
                nc.tensor.matmul(box["ps"],
                                 lhsT=wo_sb[p4][:, m * P:(m + 1) * P],
                                 rhs=an_sb[p4][:, ch * CW:(ch + 1) * CW],
                                 start=(p4 == 0), stop=(p4 == NPAIR - 1))
            def fin(box=box):
                osb = opool.tile([P, CW], F32, tag="osb", name=f"o{m}_{ch}",
                                 bufs=2)
                nc.vector.tensor_copy(osb, box["ps"])
                nc.sync.dma_start(
                    out=outT[m * P:(m + 1) * P, ch * CW:(ch + 1) * CW],
                    in_=osb)
            return [lambda p4=p4, mm=mm: mm(p4) for p4 in range(NPAIR)] + [fin]

        # ---------------- attention emission ----------------
        state = {"eb": {(0, 0): eb0}}

        def normalize(p, ch, pattn):
            saus = []
            for hf in range(2):
                sau = smpool.tile([65, CW], BF, tag="sau",
                                  name=f"sa{p}_{ch}_{hf}", bufs=3)
                saus.append(sau)
                nc.vector.tensor_copy(sau, pattn[hf])
            den_t = smpool.tile([2, CW], BF, tag="dent",
                                name=f"den{p}_{ch}", bufs=1)
            for hf in range(2):
                nc.gpsimd.dma_start(out=den_t[hf:hf + 1, :],
                                    in_=saus[hf][64:65, :])
            denf = smpool.tile([2, CW], F32, tag="denf",
                               name=f"dnf{p}_{ch}", bufs=1)
            nc.vector.tensor_copy(denf, den_t)
            recipf = smpool.tile([2, CW], F32, tag="recipf",
                                 name=f"rcf{p}_{ch}", bufs=1)
            nc.vector.reciprocal_approx_fast(recipf, denf)
            recipb = smpool.tile([2, CW], BF, tag="recipb",
                                 name=f"rcb{p}_{ch}", bufs=1)
            nc.vector.tensor_copy(recipb, recipf)
            rscr = dpool.tile([2, CW], BF, tag="rscr",
                              name=f"rs{p}_{ch}", bufs=2)
            nc.gpsimd.dma_start(out=rscr, in_=recipb)
            for hf in range(2):
                bc = smpool.tile([DEPTH, CW], BF, tag="bc",
                                 name=f"bc{p}_{ch}_{hf}", bufs=2)
                nc.gpsimd.dma_start(
                    out=bc, in_=rscr[hf:hf + 1, :].partition_broadcast(DEPTH))
                nc.vector.tensor_mul(
                    an_sb[p][hf * DEPTH:(hf + 1) * DEPTH,
                             ch * CW:(ch + 1) * CW],
                    saus[hf][0:DEPTH, :], bc)

        def run_pair(p, filler):
            """Emit one head-pair's attention with lag-LAG attn matmuls.

            filler: deque of (deadline, op) per-MM closures drained into PE
            slack; ops whose deadline (pair, chunk) has arrived are force-
            drained at chunk starts to keep emission (= dependency) order.
            """
            ha, hb = 2 * p, 2 * p + 1
            pend = deque()   # (ch, c, ew2, pattn) awaiting attn emission
            pattn_box = {}   # ch -> [pattn_h0, pattn_h1]

            def emit_attn():
                ch, c, ew2, pattn = pend.popleft()
                nc.tensor.matmul(pattn[0], lhsT=v_sb[c][:, ha, 0:65],
                                 rhs=ew2[:, 0:CW],
                                 start=(c == 0), stop=(c == KT - 1))
                nc.tensor.matmul(pattn[1], lhsT=v_sb[c][:, hb, 0:65],
                                 rhs=ew2[:, CW:2 * CW],
                                 start=(c == 0), stop=(c == KT - 1))
                if c == KT - 1:
                    normalize(p, ch, pattn)
                    del pattn_box[ch]

            for ic in range(NCH * KT):
                ch, c = divmod(ic, KT)
                if c == 0:
                    while filler and filler[0][0] <= (p, ch):
                        filler.popleft()[1]()
                    # prefetch next eb slab (2-slab ring)
                    np_, nch = (p, ch + 1) if ch + 1 < NCH else (p + 1, 0)
                    if np_ < NPAIR:
                        nxt = [eb_tile(np_, nch, cc) for cc in range(KT)]
                        dma_eb_slab(nch, nxt)
                        state["eb"][(np_, nch)] = nxt
                    pattn_box[ch] = [
                        pap.tile([65, CW], F32, tag="pattn",
                                 name=f"pa{p}_{ch}_{hf}", bufs=2)
                        for hf in range(2)]
                eb_cur = state["eb"][(p, ch)]
                plt = plp.tile([P, 2 * CW], F32, tag="pl",
                               name=f"pl{p}_{ch}_{c}", bufs=2)
                nc.tensor.matmul(plt[:, 0:CW],
                                 lhsT=kT_sb[p][0:DEPTH, c * P:(c + 1) * P],
                                 rhs=qT_sb[p][0:DEPTH, ch * CW:(ch + 1) * CW],
                                 start=True, stop=True)
                nc.tensor.matmul(plt[:, CW:2 * CW],
                                 lhsT=kT_sb[p][DEPTH:2 * DEPTH,
                                               c * P:(c + 1) * P],
                                 rhs=qT_sb[p][DEPTH:2 * DEPTH,
                                              ch * CW:(ch + 1) * CW],
                                 start=True, stop=True)
                ew = epool.tile([P, 2 * CW], BF, tag="ew",
                                name=f"ew{p}_{ch}_{c}", bufs=2)
                nc.scalar.activation(ew, plt, EXP)
                ew2 = epool.tile([P, 2 * CW], BF, tag="ew2",
                                 name=f"ew2{p}_{ch}_{c}", bufs=LAG + 2)
                nc.vector.tensor_mul(ew2[:, 0:CW], ew[:, 0:CW], eb_cur[c])
                nc.vector.tensor_mul(ew2[:, CW:2 * CW], ew[:, CW:2 * CW],
                                     eb_cur[c])
                pend.append((ch, c, ew2, pattn_box[ch]))
                if len(pend) > LAG:
                    emit_attn()
                ndrain = 9 if (p == 0 and ic < KT) else (2 + ic % 2)
                for _ in range(min(ndrain, len(filler))):
                    filler.popleft()[1]()
                if ch == NCH - 1 and c == KT - 1:
                    del state["eb"][(p, ch)]
            while pend:
                emit_attn()

        # ---- prologue compute: pair-0 k, q chunk 0, first v quarters ----
        for kkc in range(NCH):
            for op in k_group_ops(0, kkc):
                op()
        for op in q_group_ops(0, 0):
            op()
        vq0 = [v_group_ops(c, 0, 2) for c in range(KT)]
        for c in range(4):
            for op in vq0[c]:
                op()

        fill = deque()

        def add(dl, ops):
            fill.extend((dl, op) for op in ops)

        # pair-0 v quarters race the lagged chunk-0 attn (boosted drain)
        for c in range(4, KT):
            add((0, 0), vq0[c])
        add((0, 1), q_group_ops(0, 1))
        # v heads 2-7 (pairs 1-3) must land before pair 1
        for c in range(KT):
            add((1, 0), v_group_ops(c, 2, 6))
        add((0, 2), q_group_ops(0, 2))
        add((1, 0), k_group_ops(1, 0) + k_group_ops(1, 1))
        add((0, 3), q_group_ops(0, 3))
        add((1, 0), k_group_ops(1, 2) + k_group_ops(1, 3))
        add((1, 0), q_group_ops(1, 0))
        run_pair(0, fill)

        add((1, 1), q_group_ops(1, 1))
        for kkc in range(NCH):
            add((2, 0), k_group_ops(2, kkc))
        add((1, 2), q_group_ops(1, 2))
        add((2, 0), q_group_ops(2, 0))
        add((1, 3), q_group_ops(1, 3))
        run_pair(1, fill)

        add((2, 1), q_group_ops(2, 1))
        for kkc in range(NCH):
            add((3, 0), k_group_ops(3, kkc))
        add((2, 2), q_group_ops(2, 2))
        add((3, 0), [lambda p=p: load_wo(p) for p in range(NPAIR)])
        add((2, 3), q_group_ops(2, 3))
        add((3, 0), q_group_ops(3, 0))
        add((3, 1), q_group_ops(3, 1))
        run_pair(2, fill)

        add((3, 2), q_group_ops(3, 2))
        add((3, 3), q_group_ops(3, 3))
        for ch in range(NCH - 1):
            for m in range(NU):
                add((3, ch + 1), out_group_ops(m, ch))
        run_pair(3, fill)

        # tail: drain leftovers + last chunk's out-projection
        while fill:
            fill.popleft()[1]()
        for m in range(NU):
            for op in out_group_ops(m, NCH - 1):
                op()


def build_nc():
    nc = bacc.Bacc("TRN2", target_bir_lowering=False, debug=False)
    io = {
        "xT": nc.dram_tensor("xT", [D, S], BF, kind="ExternalInput").ap(),
        "yT": nc.dram_tensor("yT", [D, S], BF, kind="ExternalInput").ap(),
        "wqT": nc.dram_tensor("wqT", [D, DH], BF, kind="ExternalInput").ap(),
        "wkT": nc.dram_tensor("wkT", [D, DH], BF, kind="ExternalInput").ap(),
        "wvT": nc.dram_tensor("wvT", [D, DH], BF, kind="ExternalInput").ap(),
        "woT": nc.dram_tensor("woT", [DH, D], BF, kind="ExternalInput").ap(),
        "ebt": nc.dram_tensor("ebt", [NCH * S, CW], BF,
                              kind="ExternalInput").ap(),
        "outT": nc.dram_tensor("outT", [D, S], F32,
                               kind="ExternalOutput").ap(),
    }
    with tile.TileContext(nc) as tc:
        with ExitStack() as ctx:
            _attn_body(ctx, tc, io)
    nc.compile()
    return nc


_NC_CACHE = None


def kernel(x, y, bias, Wq, Wk, Wv, Wo):
    global _NC_CACHE, last_exec_time_ns, last_results
    x = np.asarray(x, np.float32)
    y = np.asarray(y, np.float32)
    bias = np.asarray(bias, np.float32)
    Wq, Wk, Wv, Wo = (np.asarray(w, np.float32) for w in (Wq, Wk, Wv, Wo))
    if _NC_CACHE is None:
        _NC_CACHE = build_nc()
    nc = _NC_CACHE

    bf = ml_dtypes.bfloat16
    scale = DEPTH ** -0.5
    wqT = np.ascontiguousarray(Wq.T * scale).astype(bf)
    wkT = np.ascontiguousarray(Wk.T).astype(bf)
    wvT = np.ascontiguousarray(Wv.T).astype(bf)
    woT = np.ascontiguousarray(Wo.T).astype(bf)

    # exp(bias).T pre-tiled: row ch*S + c*128 + p  <-  ebT[c*128+p, ch*512:+512]
    ebT = np.exp(bias[0, 0].astype(np.float32)).T
    ebt = np.ascontiguousarray(
        ebT.reshape(S, NCH, CW).transpose(1, 0, 2).reshape(NCH * S, CW)
    ).astype(bf)

    yT_all = [np.ascontiguousarray(y[b].T).astype(bf) for b in range(B)]
    xT_all = [np.ascontiguousarray(x[b].T).astype(bf) for b in range(B)]
    whalf = []
    for h in range(2):
        sl = slice(h * DH, (h + 1) * DH)
        whalf.append({
            "wqT": np.ascontiguousarray(wqT[:, sl]),
            "wkT": np.ascontiguousarray(wkT[:, sl]),
            "wvT": np.ascontiguousarray(wvT[:, sl]),
            "woT": np.ascontiguousarray(woT[sl, :]),
        })

    in_maps = []
    for core in range(NCORES):
        b, half = divmod(core, 2)
        m = {"xT": xT_all[b], "yT": yT_all[b], "ebt": ebt}
        m.update(whalf[half])
        in_maps.append(m)

    res = run_bass_kernel_spmd(nc, in_maps, core_ids=list(range(NCORES)),
                               trace=TRACE)
    last_exec_time_ns = res.exec_time_ns
    last_results = res
    out = np.empty((B, S, D), np.float32)
    for b in range(B):
        acc = res.results[2 * b]["outT"] + res.results[2 * b + 1]["outT"]
        out[b] = acc.T
    return out


# revision 10
# speedup vs baseline: 1.2235x; 1.0211x over previous
"""Multi-head attention with bias, distributed over 8 trn2 NeuronCores.

Reference computation (per batch b):
    q = (x @ Wq.T) * depth**-0.5 ; k = y @ Wk.T ; v = y @ Wv.T     (per-head split)
    out = softmax(q @ k.T + bias) @ v @ Wo.T

Sharding: 8 cores = 4 batches x 2 head-halves (tensor parallel over heads).
Core c handles batch b = c//2 and heads (c%2)*8 .. +8.  Wq/Wk/Wv are
column-split, Wo row-split; the two partial outputs per batch are summed on
the host (no device collective).

Device-side layout (feature dim on partitions):
    qT/kT = W.T-projected activations [d_out=512, 2048]; v natural [kk, h, d].
    logitsT[kk, q] per head via row-tiled K=64 matmul pairs (2 heads share
    the 128-partition d-tile; tile_position rows 0-63 / 64-127 concurrent)
    expw = exp(logitsT) * exp(bias).T      (exp(bias) precomputed on host,
                                            streamed per (pair, q-chunk))
    attnT_h(+denom row) = [v_h | ones].T @ expw  (K=128, denom rides row 64)
    normalize via DVE reciprocal + DMA partition-broadcast from DRAM
    outT_partial = Wo_half.T-proj of normalized attnT (summed on host).

Scheduling: attn matmuls LAG two iterations behind their logits pair so the
in-order PE queue never blocks on the ACT->DVE chain; the normalization
epilogue is split into three stages deferred across following iterations so
its DMA round-trips never head-block the DVE queue; projection matmuls are
chopped into per-MM filler ops drained into PE slack with (deadline,
earliest) emission gates.  Inputs arrive as one large striped DMA per
tensor in critical-path order.
Host does: transposes, bf16 casts, exp(bias) pre-tiling, scale fold into Wq.
"""

import numpy as np
import ml_dtypes
from collections import deque
from contextlib import ExitStack

import concourse.bass as bass
import concourse.mybir as mybir
import concourse.tile as tile
from concourse import bacc
from concourse.bass_utils import run_bass_kernel_spmd

# full-problem dims (hardcoded per spec)
B, S, D, H = 4, 2048, 1024, 16
DEPTH = D // H            # 64
P = 128
NCORES = 8

DH = D // 2               # 512 head dims per core (8 heads)
NPAIR = 4                 # head pairs per core
NCH = 4                   # q chunks of 512
KT = S // P               # 16 kk tiles
NU = D // P               # 8 d_in tiles
CW = 512                  # q chunk width
LAG = 2                   # attn matmul lag (iterations) behind logits

BF = mybir.dt.bfloat16
F32 = mybir.dt.float32
EXP = mybir.ActivationFunctionType.Exp

TRACE = False
last_exec_time_ns = None
last_results = None


def _attn_body(ctx, tc, io):
    nc = tc.nc
    xT, yT, wqT, wkT, wvT, woT, ebt, outT = (
        io[k] for k in ("xT", "yT", "wqT", "wkT", "wvT", "woT", "ebt", "outT"))

    # ---------------- persistent pools ----------------
    qpool = ctx.enter_context(tc.tile_pool(name="qpool", bufs=NPAIR))
    kpool = ctx.enter_context(tc.tile_pool(name="kpool", bufs=NPAIR))
    vpool = ctx.enter_context(tc.tile_pool(name="vpool", bufs=KT))
    anpool = ctx.enter_context(tc.tile_pool(name="anpool", bufs=NPAIR))
    ebpool = ctx.enter_context(tc.tile_pool(name="ebpool", bufs=20))
    epool = ctx.enter_context(tc.tile_pool(name="epool", bufs=6))
    smpool = ctx.enter_context(tc.tile_pool(name="smpool", bufs=4))
    plp = ctx.enter_context(tc.tile_pool(name="plp", bufs=2, space="PSUM"))
    pap = ctx.enter_context(tc.tile_pool(name="pap", bufs=2, space="PSUM"))
    pop = ctx.enter_context(tc.tile_pool(name="pop", bufs=2, space="PSUM"))
    dpool = ctx.enter_context(tc.tile_pool(name="dpool", bufs=2, space="DRAM"))
    wopool = ctx.enter_context(tc.tile_pool(name="wopool", bufs=NPAIR))
    opool = ctx.enter_context(tc.tile_pool(name="opool", bufs=2))

    qT_sb = [qpool.tile([P, S], BF, tag="qT", name=f"qT{p}", bufs=NPAIR)
             for p in range(NPAIR)]
    kT_sb = [kpool.tile([P, S], BF, tag="kT", name=f"kT{p}", bufs=NPAIR)
             for p in range(NPAIR)]
    v_sb = [vpool.tile([P, 2 * NPAIR, 66], BF, tag="v66", name=f"v{c}",
                       bufs=KT) for c in range(KT)]
    an_sb = [anpool.tile([P, S], BF, tag="an", name=f"an{p}", bufs=NPAIR)
             for p in range(NPAIR)]
    wo_sb = [wopool.tile([P, D], BF, tag="wo", name=f"wo{p}", bufs=NPAIR)
             for p in range(NPAIR)]

    def eb_tile(p, ch, c):
        return ebpool.tile([P, CW], BF, tag="eb", name=f"eb{p}_{ch}_{c}",
                           bufs=20)

    def dma_eb_slab(ch, tiles):
        base = ch * S
        for c in range(KT):
            nc.sync.dma_start(out=tiles[c],
                              in_=ebt[base + c * P:base + (c + 1) * P, :])

    # ---------------- input loads + projection helpers ----------------
    with tc.tile_pool(name="ypool", bufs=1) as ypool, \
         tc.tile_pool(name="xpool", bufs=1) as xpool, \
         tc.tile_pool(name="wkpool", bufs=1) as wkpool, \
         tc.tile_pool(name="wqpool", bufs=1) as wqpool, \
         tc.tile_pool(name="wvpool", bufs=1) as wvpool:
        wk_sb = wkpool.tile([P, NU, DH], BF, tag="wk", name="wk")
        wv_sb = wvpool.tile([P, NU, DH], BF, tag="wv", name="wv")
        y_sb = ypool.tile([P, NU, S], BF, tag="yT", name="y")
        wq_sb = wqpool.tile([P, NU, DH], BF, tag="wq", name="wq")
        x_sb = xpool.tile([P, NU, S], BF, tag="xT", name="x")
        # one large striped DMA per tensor, critical-path order on one queue
        nc.sync.dma_start(out=wk_sb,
                          in_=wkT.rearrange("(u p) j -> p u j", p=P))
        nc.sync.dma_start(out=wv_sb,
                          in_=wvT.rearrange("(u p) j -> p u j", p=P))
        nc.sync.dma_start(out=y_sb,
                          in_=yT.rearrange("(u p) j -> p u j", p=P))
        nc.sync.dma_start(out=wq_sb,
                          in_=wqT.rearrange("(u p) j -> p u j", p=P))
        nc.sync.dma_start(out=x_sb,
                          in_=xT.rearrange("(u p) j -> p u j", p=P))
        # eb slab 0 queued after x: first tiles land right as ACT starts
        eb0 = [eb_tile(0, 0, c) for c in range(KT)]
        dma_eb_slab(0, eb0)

        # warm-up heartbeats chained to arriving inputs keep the PE HAM
        # activity window alive through the load phase
        jnk0 = plp.tile([1, 1024], F32, tag="pl", name="jnk0", bufs=2)
        for t in (wk_sb, wv_sb, y_sb):
            nc.tensor.matmul(jnk0[0:1, 0:512], lhsT=t[0:1, 0, 0:1],
                             rhs=t[0:1, 0, 0:512], start=True, stop=True)

        # ---- per-MM filler ops ----
        def k_group_ops(p, kkc):
            box = {}
            def mm(u, box=box):
                if u == 0:
                    box["ps"] = pop.tile([P, CW], F32, tag="po",
                                         name=f"psk{p}_{kkc}", bufs=2)
                nc.tensor.matmul(box["ps"],
                                 lhsT=wk_sb[:, u, p * P:(p + 1) * P],
                                 rhs=y_sb[:, u, kkc * CW:(kkc + 1) * CW],
                                 start=(u == 0), stop=(u == NU - 1))
            def fin(box=box):
                nc.vector.tensor_copy(
                    kT_sb[p][:, kkc * CW:(kkc + 1) * CW], box["ps"])
            return [lambda u=u, mm=mm: mm(u) for u in range(NU)] + [fin]

        def q_group_ops(p, ch):
            box = {}
            def mm(u, box=box):
                if u == 0:
                    box["ps"] = pop.tile([P, CW], F32, tag="po",
                                         name=f"psq{p}_{ch}", bufs=2)
                nc.tensor.matmul(box["ps"],
                                 lhsT=wq_sb[:, u, p * P:(p + 1) * P],
                                 rhs=x_sb[:, u, ch * CW:(ch + 1) * CW],
                                 start=(u == 0), stop=(u == NU - 1))
            def fin(box=box):
                nc.vector.tensor_copy(
                    qT_sb[p][:, ch * CW:(ch + 1) * CW], box["ps"])
            return [lambda u=u, mm=mm: mm(u) for u in range(NU)] + [fin]

        def v_group_ops(c):
            box = {}
            def mm(u, box=box):
                if u == 0:
                    box["ps"] = pop.tile([P, CW], F32, tag="po",
                                         name=f"psv{c}", bufs=2)
                nc.tensor.matmul(box["ps"],
                                 lhsT=y_sb[:, u, c * P:(c + 1) * P],
                                 rhs=wv_sb[:, u, 0:DH],
                                 start=(u == 0), stop=(u == NU - 1))
            def fin(box=box):
                vt = v_sb[c]
                nc.vector.memset(vt[:, :, 64:65], 1.0)
                nc.vector.memset(vt[:, :, 65:66], 0.0)
                src = box["ps"].rearrange("p (g d) -> p g d", d=DEPTH)
                nc.vector.tensor_copy(vt[:, :, 0:DEPTH], src)
            return [lambda u=u, mm=mm: mm(u) for u in range(NU)] + [fin]

        def load_wo(p):
            nc.gpsimd.dma_start(out=wo_sb[p], in_=woT[p * P:(p + 1) * P, :])

        def out_group_ops(m, ch):
            box = {}
            def mm(p4, box=box):
                if p4 == 0:
                    box["ps"] = pop.tile([P, CW], F32, tag="po",
                                         name=f"pso{m}_{ch}", bufs=2)
                nc.tensor.matmul(box["ps"],
                                 lhsT=wo_sb[p4][:, m * P:(m + 1) * P],
                                 rhs=an_sb[p4][:, ch * CW:(ch + 1) * CW],
                                 start=(p4 == 0), stop=(p4 == NPAIR - 1))
            def fin(box=box):
                osb = opool.tile([P, CW], F32, tag="osb", name=f"o{m}_{ch}",
                                 bufs=2)
                nc.vector.tensor_copy(osb, box["ps"])
                nc.sync.dma_start(
                    out=outT[m * P:(m + 1) * P, ch * CW:(ch + 1) * CW],
                    in_=osb)
            return [lambda p4=p4, mm=mm: mm(p4) for p4 in range(NPAIR)] + [fin]

        # ---------------- attention emission ----------------
        state = {"eb": {(0, 0): eb0}}

        def norm_a(p, ch, pattn):
            """Stage A (at last attn MM): drain psum, gather denominators."""
            saus = []
            for hf in range(2):
                sau = smpool.tile([65, CW], BF, tag="sau",
                                  name=f"sa{p}_{ch}_{hf}", bufs=3)
                saus.append(sau)
                nc.vector.tensor_copy(sau, pattn[hf])
            den_t = smpool.tile([2, CW], BF, tag="dent",
                                name=f"den{p}_{ch}", bufs=1)
            for hf in range(2):
                nc.gpsimd.dma_start(out=den_t[hf:hf + 1, :],
                                    in_=saus[hf][64:65, :])
            return saus, den_t

        def norm_b(p, ch, den_t):
            """Stage B (+1 iter): reciprocal + DRAM broadcast round-trip."""
            denf = smpool.tile([2, CW], F32, tag="denf",
                               name=f"dnf{p}_{ch}", bufs=1)
            nc.vector.tensor_copy(denf, den_t)
            recipf = smpool.tile([2, CW], F32, tag="recipf",
                                 name=f"rcf{p}_{ch}", bufs=1)
            nc.vector.reciprocal_approx_fast(recipf, denf)
            recipb = smpool.tile([2, CW], BF, tag="recipb",
                                 name=f"rcb{p}_{ch}", bufs=1)
            nc.vector.tensor_copy(recipb, recipf)
            rscr = dpool.tile([2, CW], BF, tag="rscr",
                              name=f"rs{p}_{ch}", bufs=2)
            nc.gpsimd.dma_start(out=rscr, in_=recipb)
            bcs = []
            for hf in range(2):
                bc = smpool.tile([DEPTH, CW], BF, tag="bc",
                                 name=f"bc{p}_{ch}_{hf}", bufs=2)
                bcs.append(bc)
                nc.gpsimd.dma_start(
                    out=bc, in_=rscr[hf:hf + 1, :].partition_broadcast(DEPTH))
            return bcs

        def norm_c(p, ch, saus, bcs):
            """Stage C (+3 iters): apply reciprocal, write an_sb."""
            for hf in range(2):
                nc.vector.tensor_mul(
                    an_sb[p][hf * DEPTH:(hf + 1) * DEPTH,
                             ch * CW:(ch + 1) * CW],
                    saus[hf][0:DEPTH, :], bcs[hf])

        def run_pair(p, filler):
            """Emit one head-pair's attention with lag-LAG attn matmuls.

            filler: deque of (deadline, earliest, op); ops drain into PE
            slack once `earliest` (pair, ic) has passed, and are force-
            drained when their deadline (pair, chunk) arrives.
            """
            ha, hb = 2 * p, 2 * p + 1
            pend = deque()   # (ch, c, ew2, pattn) awaiting attn emission
            pattn_box = {}
            late = {}        # ic -> [deferred closures]

            def emit_attn(ic):
                ch, c, ew2, pattn = pend.popleft()
                nc.tensor.matmul(pattn[0], lhsT=v_sb[c][:, ha, 0:65],
                                 rhs=ew2[:, 0:CW],
                                 start=(c == 0), stop=(c == KT - 1))
                nc.tensor.matmul(pattn[1], lhsT=v_sb[c][:, hb, 0:65],
                                 rhs=ew2[:, CW:2 * CW],
                                 start=(c == 0), stop=(c == KT - 1))
                if c == KT - 1:
                    saus, den_t = norm_a(p, ch, pattn)
                    del pattn_box[ch]
                    box = {}
                    def stage_b(box=box, den_t=den_t):
                        box["bcs"] = norm_b(p, ch, den_t)
                    def stage_c(box=box, saus=saus):
                        norm_c(p, ch, saus, box["bcs"])
                    late.setdefault(ic + 1, []).append(stage_b)
                    late.setdefault(ic + 3, []).append(stage_c)

            for ic in range(NCH * KT):
                ch, c = divmod(ic, KT)
                for op in late.pop(ic, ()):
                    op()
                if c == 0:
                    while filler and filler[0][0] <= (p, ch):
                        filler.popleft()[2]()
                    # prefetch next eb slab (ring of 20 tiles)
                    np_, nch = (p, ch + 1) if ch + 1 < NCH else (p + 1, 0)
                    if np_ < NPAIR:
                        nxt = [eb_tile(np_, nch, cc) for cc in range(KT)]
                        dma_eb_slab(nch, nxt)
                        state["eb"][(np_, nch)] = nxt
                    pattn_box[ch] = [
                        pap.tile([65, CW], F32, tag="pattn",
                                 name=f"pa{p}_{ch}_{hf}", bufs=2)
                        for hf in range(2)]
                eb_cur = state["eb"][(p, ch)]
                plt = plp.tile([P, 2 * CW], F32, tag="pl",
                               name=f"pl{p}_{ch}_{c}", bufs=2)
                nc.tensor.matmul(plt[:, 0:CW],
                                 lhsT=kT_sb[p][0:DEPTH, c * P:(c + 1) * P],
                                 rhs=qT_sb[p][0:DEPTH, ch * CW:(ch + 1) * CW],
                                 start=True, stop=True)
                nc.tensor.matmul(plt[:, CW:2 * CW],
                                 lhsT=kT_sb[p][DEPTH:2 * DEPTH,
                                               c * P:(c + 1) * P],
                                 rhs=qT_sb[p][DEPTH:2 * DEPTH,
                                              ch * CW:(ch + 1) * CW],
                                 start=True, stop=True)
                ew = epool.tile([P, 2 * CW], BF, tag="ew",
                                name=f"ew{p}_{ch}_{c}", bufs=2)
                nc.scalar.activation(ew, plt, EXP)
                ew2 = epool.tile([P, 2 * CW], BF, tag="ew2",
                                 name=f"ew2{p}_{ch}_{c}", bufs=LAG + 2)
                nc.vector.tensor_mul(ew2[:, 0:CW], ew[:, 0:CW], eb_cur[c])
                nc.vector.tensor_mul(ew2[:, CW:2 * CW], ew[:, CW:2 * CW],
                                     eb_cur[c])
                pend.append((ch, c, ew2, pattn_box[ch]))
                if len(pend) > LAG:
                    emit_attn(ic)
                ndrain = 9 if ic < KT else (2 + ic % 2)
                while ndrain > 0 and filler and filler[0][1] <= (p, ic):
                    filler.popleft()[2]()
                    ndrain -= 1
                if ch == NCH - 1 and c == KT - 1:
                    del state["eb"][(p, ch)]
            while pend:
                emit_attn(NCH * KT)
            for ic2 in sorted(late):
                for op in late[ic2]:
                    op()

        # ---- prologue compute: fills the x-DMA wait window ----
        for kkc in (0, 1):
            for op in k_group_ops(0, kkc):
                op()
        for c in range(6):
            for op in v_group_ops(c):
                op()
        for kkc in (2, 3):
            for op in k_group_ops(0, kkc):
                op()
        for op in q_group_ops(0, 0):
            op()

        fill = deque()
        ANY = (-1, -1)

        def add(dl, ops, earliest=ANY):
            fill.extend((dl, earliest, op) for op in ops)

        for c in range(6, KT):
            add((0, 1), v_group_ops(c))
        add((0, 1), q_group_ops(0, 1))
        for kkc in range(NCH):
            add((1, 0), k_group_ops(1, kkc))
        add((0, 2), q_group_ops(0, 2))
        add((1, 0), q_group_ops(1, 0))
        add((0, 3), q_group_ops(0, 3))
        run_pair(0, fill)

        add((1, 1), q_group_ops(1, 1))
        for kkc in range(NCH):
            add((2, 0), k_group_ops(2, kkc))
        add((1, 2), q_group_ops(1, 2))
        add((2, 0), q_group_ops(2, 0))
        add((1, 3), q_group_ops(1, 3))
        run_pair(1, fill)

        add((2, 1), q_group_ops(2, 1))
        for kkc in range(NCH):
            add((3, 0), k_group_ops(3, kkc))
        add((2, 2), q_group_ops(2, 2))
        add((3, 0), q_group_ops(3, 0))
        add((2, 3), q_group_ops(2, 3))
        add((3, 0), [lambda p=p: load_wo(p) for p in range(NPAIR)])
        add((3, 1), q_group_ops(3, 1))
        run_pair(2, fill)

        add((3, 2), q_group_ops(3, 2))
        add((3, 3), q_group_ops(3, 3))
        for ch in range(NCH - 1):
            for m in range(NU):
                add((3, ch + 2) if ch + 2 < NCH else (3, 3),
                    out_group_ops(m, ch),
                    earliest=(3, (ch + 1) * KT + 5))
        run_pair(3, fill)

        # tail: drain leftovers + last chunk's out-projection
        while fill:
            fill.popleft()[2]()
        for m in range(NU):
            for op in out_group_ops(m, NCH - 1):
                op()


def build_nc():
    nc = bacc.Bacc("TRN2", target_bir_lowering=False, debug=False)
    io = {
        "xT": nc.dram_tensor("xT", [D, S], BF, kind="ExternalInput").ap(),
        "yT": nc.dram_tensor("yT", [D, S], BF, kind="ExternalInput").ap(),
        "wqT": nc.dram_tensor("wqT", [D, DH], BF, kind="ExternalInput").ap(),
        "wkT": nc.dram_tensor("wkT", [D, DH], BF, kind="ExternalInput").ap(),
        "wvT": nc.dram_tensor("wvT", [D, DH], BF, kind="ExternalInput").ap(),
        "woT": nc.dram_tensor("woT", [DH, D], BF, kind="ExternalInput").ap(),
        "ebt": nc.dram_tensor("ebt", [NCH * S, CW], BF,
                              kind="ExternalInput").ap(),
        "outT": nc.dram_tensor("outT", [D, S], F32,
                               kind="ExternalOutput").ap(),
    }
    with tile.TileContext(nc) as tc:
        with ExitStack() as ctx:
            _attn_body(ctx, tc, io)
    nc.compile()
    return nc


_NC_CACHE = None


def kernel(x, y, bias, Wq, Wk, Wv, Wo):
    global _NC_CACHE, last_exec_time_ns, last_results
    x = np.asarray(x, np.float32)
    y = np.asarray(y, np.float32)
    bias = np.asarray(bias, np.float32)
    Wq, Wk, Wv, Wo = (np.asarray(w, np.float32) for w in (Wq, Wk, Wv, Wo))
    if _NC_CACHE is None:
        _NC_CACHE = build_nc()
    nc = _NC_CACHE

    bf = ml_dtypes.bfloat16
    scale = DEPTH ** -0.5
    wqT = np.ascontiguousarray(Wq.T * scale).astype(bf)
    wkT = np.ascontiguousarray(Wk.T).astype(bf)
    wvT = np.ascontiguousarray(Wv.T).astype(bf)
    woT = np.ascontiguousarray(Wo.T).astype(bf)

    # exp(bias).T pre-tiled: row ch*S + c*128 + p  <-  ebT[c*128+p, ch*512:+512]
    ebT = np.exp(bias[0, 0].astype(np.float32)).T
    ebt = np.ascontiguousarray(
        ebT.reshape(S, NCH, CW).transpose(1, 0, 2).reshape(NCH * S, CW)
    ).astype(bf)

    yT_all = [np.ascontiguousarray(y[b].T).astype(bf) for b in range(B)]
    xT_all = [np.ascontiguousarray(x[b].T).astype(bf) for b in range(B)]
    whalf = []
    for h in range(2):
        sl = slice(h * DH, (h + 1) * DH)
        whalf.append({
            "wqT": np.ascontiguousarray(wqT[:, sl]),
            "wkT": np.ascontiguousarray(wkT[:, sl]),
            "wvT": np.ascontiguousarray(wvT[:, sl]),
            "woT": np.ascontiguousarray(woT[sl, :]),
        })

    in_maps = []
    for core in range(NCORES):
        b, half = divmod(core, 2)
        m = {"xT": xT_all[b], "yT": yT_all[b], "ebt": ebt}
        m.update(whalf[half])
        in_maps.append(m)

    res = run_bass_kernel_spmd(nc, in_maps, core_ids=list(range(NCORES)),
                               trace=TRACE)
    last_exec_time_ns = res.exec_time_ns
    last_results = res
    out = np.empty((B, S, D), np.float32)
    for b in range(B):
        acc = res.results[2 * b]["outT"] + res.results[2 * b + 1]["outT"]
        out[b] = acc.T
    return out
